# revision 16
# baseline (speedup 1.0000x reference)
"""Trainium2 Bass kernel for nn_AgeUGP_v2 (gnn_message_passing).

Reference pipeline:
  snp_h[b,n,f] = snp[b,n] * filters[f,n]
  gathered     = snp_h[:, snp_ids, :]
  per_gene     = segment_sum(gathered, node_seg)   # node_seg sorted
  sample_h     = per_gene.mean(-1)
  h1 = sample_h @ W1 ... tiny MLP tail

Algebraic collapse: the filter axis F is only averaged at the end, so
  sample_h[b,g] = sum_{i in seg g} snp[b, id_i] * fbar[id_i],
  fbar = mean(filters, axis=0).

Device strategy v3 (8 NeuronCores, genes sharded across cores):
  - Per-core SNP COMPACTION: each core's nodes reference ~197k unique SNPs
    (of 500k); the host selects and orders just those (pure permutation),
    split into 64 chunks of Kc.  4 table passes; pass T holds 16 chunks on
    128 partitions: partition p = 16g + 8h + b carries chunk 16T+g+8h,
    batch b.
  - PACKED table DMA + parity-zeroed route: the host ships only the raw
    chunk data [128, Kc] per pass (half the bytes of the old zero-split
    layout); it lands in the B-half region [Kc:2Kc) of a persistent wide
    table [128, 2Kc+2].  fbar is produced fused on device: bf16 filters are
    hit with 1/8-valued mean+replicate PE matmuls whose routeA (routeB)
    matrices are ZERO on h=1 (h=0) partition columns.  The A-half multiply
    vtab[:,0:Kc) = raw * prA then writes data*fbar on h=0 lanes and EXACT 0
    on h=1 lanes; the B-half multiply runs in place.  The resulting table
    has the zero-split property (an index in [0,Kc) reads chunk A's value
    on h=0 lanes and 0 on h=1 lanes) without shipping or memsetting zeros
    (the two trailing zero columns are memset once).
  - One gpsimd ap_gather per pass streams both chunks' nodes gene-ordered
    (per-gene counts padded to EVEN with pads pointing at the zero column).
    A DVE tensor_tensor_scan with data0/data1 = even/odd stride-2 views
    forms PAIR prefix sums in place; a second ap_gather extracts one prefix
    per gene END; one adjacent difference gives per-(gene,half,batch) sums
    in dd (bf16); sel matmuls fold halves+lanes into pst [gene, batch].
  - PER-PASS W1 ACCUMULATION: each pass's partial pst is copied to bf16 on
    the otherwise-idle Activation engine and immediately matmul'd with the
    resident W1 shard, accumulating in two PSUM banks across all 4 passes
    (h1 = sum_p partial_p @ W1).  The old end-of-kernel W1 burst is gone.
  - The FINAL pass's extraction is split into 4 gene-quarter windows so
    the last sub/selmm/W1 chain covers only ~1/4 of the genes.
  - Pass-0 table multiplies are split DVE/Pool so the first gather starts
    ~4us earlier (Pool is idle in the head anyway).
Scheduling: the Tile scheduler's internal timing model mispredicts DMA
completion, so the per-engine order is fully pinned with nosync chains.
"""

import numpy as np

B = 8
N_SNPS = 500000
N_NODES = 2000000
N_GENES = 20000
N_FILT = 8
N_CORES = 8
BN_EPS = 1e-5

_P = 128
_NCHUNK = 64  # compact SNP chunks per core
_NTAB = 4  # table passes
_EPAD = 16
_NQ = 4  # gene quarters for the final pass's split extraction


def make_cfg(Kc, J, qwin=None, n_genes=N_GENES, n_cores=N_CORES, d1=1024):
    gpc = n_genes // n_cores
    jt = -(-gpc // _P)
    gpad = jt * _P
    ns = gpc + 1  # boundaries: dummy zero + one end per gene
    nspad = -(-ns // _EPAD) * _EPAD
    # final pass: gene quarters on sel-tile boundaries (tiles of 128 genes)
    tq = jt // _NQ
    gq = [min(q * tq * _P, gpc) for q in range(_NQ)] + [gpc]
    naq = [-(-(gq[q + 1] - gq[q] + 1) // _EPAD) * _EPAD for q in range(_NQ)]
    assert J % 16 == 0 and J % 4 == 0
    assert 2 * Kc + 2 <= 2**15, "gather table exceeds num_elems limit"
    assert J <= 32752, "stream length exceeds int16 index range"
    return dict(
        Kc=Kc, J=J, gpc=gpc, gpad=gpad, jt=jt, d1=d1, ns=ns, nspad=nspad,
        n_cores=n_cores, gq=gq, naq=naq, qwin=qwin,
    )


# ---------------------------------------------------------------- device program
def build_program(cfg):
    import concourse.bass as bass
    import concourse.bacc as bacc
    import concourse.mybir as mybir
    import concourse.tile as tile

    fp32 = mybir.dt.float32
    bf16 = mybir.dt.bfloat16
    i16 = mybir.dt.int16

    Kc, J = cfg["Kc"], cfg["J"]
    jt, d1 = cfg["jt"], cfg["d1"]
    gpc, gpad, nspad = cfg["gpc"], cfg["gpad"], cfg["nspad"]
    gq, naq, qwin = cfg["gq"], cfg["naq"], cfg["qwin"]
    TW = 2 * Kc + 2  # table width: [A-half | B-half | zero col pair]
    JH = J // 2
    NS2 = sum(naq)

    nc = bacc.Bacc(
        "TRN2", target_bir_lowering=False, debug=False, num_devices=cfg["n_cores"]
    )

    snp_in = nc.dram_tensor("snp_perm", [_P, _NTAB * Kc], fp32, kind="ExternalInput")
    filt_in = nc.dram_tensor("filt_perm", [_P, _NTAB * Kc], bf16, kind="ExternalInput")
    gidx_in = nc.dram_tensor("gidx", [_P, _NTAB * (J // 16)], i16, kind="ExternalInput")
    eidx_in = nc.dram_tensor(
        "eidx", [_P, (_NTAB - 1) * (nspad // 16) + NS2 // 16], i16,
        kind="ExternalInput",
    )
    sel_in = nc.dram_tensor("sel", [_P, 8], bf16, kind="ExternalInput")
    route_in = nc.dram_tensor("mroute", [_P, 2 * _P], bf16, kind="ExternalInput")
    w1_in = nc.dram_tensor("w1c", [_P, jt * d1], bf16, kind="ExternalInput")
    h1_out = nc.dram_tensor("h1p", [B, d1], fp32, kind="ExternalOutput")

    rc = Kc // 8  # route/mul block width (single-bank PSUM tiles)
    assert rc * 8 == Kc and rc <= 512

    with tile.TileContext(nc) as tc:
        with (
            tc.tile_pool(name="per", bufs=1) as perpool,
            tc.tile_pool(name="gs", bufs=2) as gspool,
            tc.tile_pool(name="ft", bufs=2) as ftpool,
            tc.tile_pool(name="ex", bufs=2) as expool,
            tc.tile_pool(name="ixg", bufs=3) as ixgpool,
            tc.tile_pool(name="ixe", bufs=3) as ixepool,
            tc.tile_pool(name="w1", bufs=5) as w1pool,
            tc.tile_pool(name="shb", bufs=2) as shbpool,
            tc.tile_pool(name="ps", bufs=5, space="PSUM") as pspool,
            tc.tile_pool(name="psw", bufs=1, space="PSUM") as pswpool,
            tc.tile_pool(name="psh", bufs=1, space="PSUM") as pshpool,
        ):
            DMAQ, DVEQ, POOLQ, PEQ, ACTQ = [], [], [], [], []

            route = perpool.tile([_P, 2 * _P], bf16, tag="route")
            route_d = nc.sync.dma_start(route[:], route_in.ap())
            sel8 = perpool.tile([_P, 8], bf16, tag="sel8")
            sel_d = nc.sync.dma_start(sel8[:], sel_in.ap())

            # dd holds per-(lane,gene) sums; pad cols stay zero forever
            dd = perpool.tile([_P, gpad], bf16, tag="dd")
            DVEQ.append(nc.vector.memset(dd[:], 0.0))
            # two persistent wide tables; only the trailing zero column pair
            # needs initialization (the A/B data halves are regenerated by
            # the parity-zeroed multiplies each pass)
            vtabs_b = []
            for t in range(2):
                vt = perpool.tile([_P, TW], fp32, tag=f"vtab{t}")
                DVEQ.append(nc.vector.memset(vt[:, 2 * Kc : TW], 0.0))
                vtabs_b.append(vt)

            from concourse.instruction_name_ordered_set import (
                InstructionNameOrderedSet,
            )

            def pin(later, earlier):
                """Same-engine order pin (no runtime semaphore)."""
                s = InstructionNameOrderedSet()
                s.add(earlier.ins.name)
                later.ins.add_nosync_dependencies_from(s)

            def emit_table(T, pool_blocks=()):
                # raw packed data lands in the B-half region [Kc:2Kc); the
                # A-half multiply reads it (cross-region), the B-half
                # multiply runs in place.  prA/prB are zero on wrong-parity
                # partitions, so both halves get the zero-split layout
                # without any zero DMA/memset.  pool_blocks run their
                # multiplies on the (head-idle) gpsimd engine; their prs are
                # emitted first so the PE queue serves them first.
                vtab = vtabs_b[T % 2]
                ft = ftpool.tile([_P, Kc], bf16, tag="ftl", name=f"ftl{T}")
                ftd = nc.sync.dma_start(
                    ft[:], filt_in.ap()[:, T * Kc : (T + 1) * Kc]
                )
                if T == 0:
                    # lead-in: quarter the transfer so multiplies pipeline
                    KH = Kc // 4
                    qs = [
                        nc.sync.dma_start(
                            vtab[:, Kc + q * KH : Kc + (q + 1) * KH],
                            snp_in.ap()[:, q * KH : (q + 1) * KH],
                        )
                        for q in range(4)
                    ]
                    dmas = [ftd] + qs
                else:
                    dmas = [
                        ftd,
                        nc.sync.dma_start(
                            vtab[:, Kc : 2 * Kc],
                            snp_in.ap()[:, T * Kc : (T + 1) * Kc],
                        ),
                    ]
                dve_muls, pool_muls, prs = [], [], []
                blk_order = list(pool_blocks) + [
                    b for b in range(8) if b not in pool_blocks
                ]
                for blk in blk_order:
                    rs = slice(Kc + blk * rc, Kc + (blk + 1) * rc)
                    on_pool = blk in pool_blocks
                    for half in range(2):
                        pr = pspool.tile([_P, rc], fp32, tag="pr", name="pr")
                        prs.append(
                            nc.tensor.matmul(
                                pr[:],
                                route[:, half * _P : (half + 1) * _P],
                                ft[:, blk * rc : (blk + 1) * rc],
                                start=True, stop=True,
                            )
                        )
                        ks = rs if half else slice(blk * rc, (blk + 1) * rc)
                        eng = nc.gpsimd if on_pool else nc.vector
                        m = eng.tensor_mul(vtab[:, ks], vtab[:, rs], pr[:])
                        (pool_muls if on_pool else dve_muls).append(m)
                vtabs_for_pass[T] = vtab
                return dict(dmas=dmas, muls=dve_muls, pool_muls=pool_muls, prs=prs)

            vtabs_for_pass = {}

            # index streams prefetched once (each pass's stream is its own
            # tile: ap_gather idx APs must start at a tile base)
            gidx_t, eidx_t = {}, {}

            def prefetch_gidx(p):
                g = ixgpool.tile([_P, J // 16], i16, tag="gidxp", name=f"gidx{p}")
                d = nc.sync.dma_start(
                    g[:], gidx_in.ap()[:, p * (J // 16) : (p + 1) * (J // 16)]
                )
                gidx_t[p] = g
                return [d]

            def prefetch_eidx(p):
                dmas = []
                if p < _NTAB - 1:
                    e = ixepool.tile([_P, nspad // 16], i16, tag="eidxp",
                                     name=f"eidx{p}")
                    dmas.append(
                        nc.sync.dma_start(
                            e[:],
                            eidx_in.ap()[
                                :, p * (nspad // 16) : (p + 1) * (nspad // 16)
                            ],
                        )
                    )
                    eidx_t[p] = e
                else:
                    base3 = (_NTAB - 1) * (nspad // 16)
                    eqs = []
                    off = base3
                    for q in range(_NQ):
                        eq = perpool.tile([_P, naq[q] // 16], i16, tag=f"eidxq{q}")
                        dmas.append(
                            nc.sync.dma_start(
                                eq[:], eidx_in.ap()[:, off : off + naq[q] // 16]
                            )
                        )
                        eqs.append(eq)
                        off += naq[q] // 16
                    eidx_t[p] = eqs
                return dmas

            def emit_gather(p):
                gidx = gidx_t[p]
                gout = gspool.tile([_P, J], fp32, tag="gout", name=f"gout{p}")
                g1 = nc.gpsimd.ap_gather(
                    gout[:], vtabs_for_pass.pop(p)[:], gidx[:],
                    channels=_P, num_elems=TW, d=1, num_idxs=J,
                )
                return gout, g1

            def _scan_piece(gout, c0, c1, initial):
                # pair prefix over stream slots [2*c0, 2*c1) into pair cols
                # [c0, c1), chained via `initial`
                ge = gout[:, 2 * c0 :]
                even = bass.AP(ge.tensor, ge.offset, [ge.ap[0], [2, c1 - c0]])
                go = gout[:, 2 * c0 + 1 :]
                odd = bass.AP(go.tensor, go.offset, [go.ap[0], [2, c1 - c0]])
                return nc.vector.tensor_tensor_scan(
                    gout[:, c0:c1], even, odd, initial,
                    op0=mybir.AluOpType.add, op1=mybir.AluOpType.add,
                )

            def emit_scan_extract(p, gout):
                # pair prefix scan, in place into the first half (writes
                # trail the stride-2 reads)
                if p < _NTAB - 1:
                    sc = [_scan_piece(gout, 0, JH, 0.0)]
                    ext = expool.tile([_P, nspad], fp32, tag="ex", name=f"ex{p}")
                    eidx = eidx_t[p]
                    g2 = [
                        nc.gpsimd.ap_gather(
                            ext[:], gout[:, :JH], eidx[:],
                            channels=_P, num_elems=JH, d=1, num_idxs=nspad,
                        )
                    ]
                    return sc, g2, [ext]
                # final pass: the scan is chunked at the quarter windows'
                # upper bounds and each quarter extracts into its OWN tile,
                # so quarter q's reduce chain starts as soon as scan chunk q
                # and its (windowed) extraction are done
                eqs = eidx_t[p]
                sc, g2, exts = [], [], []
                c0 = 0
                for q in range(_NQ):
                    c1 = qwin[q][1] if q < _NQ - 1 else JH
                    sc.append(
                        _scan_piece(gout, c0, c1,
                                    0.0 if q == 0 else gout[:, c0 - 1 : c0])
                    )
                    lo, hi = qwin[q]
                    ext = expool.tile([_P, naq[q]], fp32, tag="ex",
                                      name=f"exq{q}")
                    g2.append(
                        nc.gpsimd.ap_gather(
                            ext[:], gout[:, lo:hi], eqs[q][:],
                            channels=_P, num_elems=hi - lo, d=1,
                            num_idxs=naq[q],
                        )
                    )
                    exts.append(ext)
                    c0 = c1
                return sc, g2, exts

            # ---- reduce: sel matmuls ACCUMULATE all 4 passes into one
            # persistent PSUM tile (start on pass 0, stop in the final
            # pass's quarter); W1 then runs once, per gene quarter --------
            n_half = min(512, d1)
            n_banks = -(-d1 // n_half)
            pss = []
            for nb in range(n_banks):
                ps = pswpool.tile([_P, n_half], fp32, tag=f"ps{nb}", name=f"ps{nb}")
                pss.append(ps)
            pst = pshpool.tile([_P, jt * B], fp32, tag="pst")

            w1ts, w1dmas = [], []
            wgrp = 4 if jt % 4 == 0 else 1  # K-tiles per W1 load
            for jg in range(jt // wgrp):
                w1t = w1pool.tile([_P, wgrp * d1], bf16, tag="w1t", name=f"w1t{jg}")
                w1dmas.append(
                    nc.sync.dma_start(
                        w1t[:], w1_in.ap()[:, jg * wgrp * d1 : (jg + 1) * wgrp * d1]
                    )
                )
                w1ts.append(w1t)

            def w1_tile(j):
                return w1ts[j // wgrp], (j % wgrp) * d1

            def emit_reduce(p, ex, tlo, thi, xoff, first, last):
                """sub -> sel matmuls accumulating pst[:, tile cols]; on the
                final pass (last=True) also bf16-copy the finished quarter
                and run its W1 matmuls (accumulating into pss banks)."""
                glo, ghi = tlo * _P, min(thi * _P, gpc)
                sub = nc.vector.tensor_sub(
                    dd[:, glo:ghi],
                    ex[:, xoff + 1 : xoff + 1 + (ghi - glo)],
                    ex[:, xoff : xoff + (ghi - glo)],
                )
                mms = []
                for t in range(tlo, thi):
                    mms.append(
                        nc.tensor.matmul(
                            pst[:, t * B : (t + 1) * B],
                            dd[:, t * _P : (t + 1) * _P],
                            sel8[:],
                            start=first, stop=last,
                        )
                    )
                if not last:
                    return dict(sub=sub, mms=mms, cp=None, wmms=[])
                shb = shbpool.tile([_P, (thi - tlo) * B], bf16, tag="shbp",
                                   name=f"shbq{tlo}")
                cp = nc.scalar.copy(shb[:], pst[:, tlo * B : thi * B])
                wmms = []
                for nb in range(n_banks):  # bank-major: bank0 stops first
                    for i, t in enumerate(range(tlo, thi)):
                        w1t, woff = w1_tile(t)
                        wmms.append(
                            nc.tensor.matmul(
                                pss[nb][:B, :],
                                shb[:, i * B : (i + 1) * B],
                                w1t[:, woff + nb * n_half : woff + (nb + 1) * n_half],
                                start=tlo == 0 and t == tlo,
                                stop=t == jt - 1,
                            )
                        )
                return dict(sub=sub, mms=mms, cp=cp, wmms=wmms)

            # ---------------- emission (dataflow order: the tile
            # framework's auto-deps are derived from emission order, so
            # writers of reused tiles must be emitted after their prior
            # readers; DMAQ/engine pins control actual timing) ----------
            tabs = {0: emit_table(0, pool_blocks=(0, 1, 2))}
            idx0 = prefetch_gidx(0)
            tabs[1] = emit_table(1)
            idx1 = prefetch_gidx(1) + prefetch_eidx(0)
            idx2 = prefetch_gidx(2) + prefetch_eidx(1)
            idx3 = prefetch_eidx(2) + prefetch_eidx(3)

            gouts, g1i, scans, g2i, exs = {}, {}, {}, {}, {}
            reds = {}
            gouts[0], g1i[0] = emit_gather(0)
            for p in range(_NTAB):
                if p == 0:
                    # gidx3 reuses gidx0's pool slot: emit after gather-0
                    idx3 = prefetch_gidx(3) + idx3
                if p + 1 < _NTAB:
                    gouts[p + 1], g1i[p + 1] = emit_gather(p + 1)
                scans[p], g2i[p], exs[p] = emit_scan_extract(p, gouts.pop(p))
                if p + 2 < _NTAB:
                    tabs[p + 2] = emit_table(p + 2)
                if p >= 1:
                    pp = p - 1
                    reds[pp] = [emit_reduce(pp, exs.pop(pp)[0], 0, jt, 0,
                                            first=pp == 0, last=False)]
            # final pass: per-quarter reduce chains
            ex3s = exs.pop(_NTAB - 1)
            reds[_NTAB - 1] = []
            tq = jt // _NQ
            for q in range(_NQ):
                reds[_NTAB - 1].append(
                    emit_reduce(_NTAB - 1, ex3s[q], q * tq,
                                min((q + 1) * tq, jt), 0,
                                first=False, last=True)
                )

            # per-bank copy + output DMA so bank 0's drain overlaps bank 1's
            # matmuls
            h1 = perpool.tile([B, d1], fp32, tag="h1")
            h1copies, out_dmas = [], []
            for nb in range(n_banks):
                h1copies.append(
                    nc.scalar.copy(
                        h1[:, nb * n_half : (nb + 1) * n_half], pss[nb][:B, :]
                    )
                )
                out_dmas.append(
                    nc.sync.dma_start(
                        h1_out.ap()[:, nb * n_half : (nb + 1) * n_half],
                        h1[:, nb * n_half : (nb + 1) * n_half],
                    )
                )

            # ---------------- static order pins ----------------
            def chain(seq):
                for a, b in zip(seq, seq[1:]):
                    pin(b, a)

            # DMA: route/sel tiny and first (they gate the route matmuls),
            # table 0 (ft first), gather-0 indices, table 1, then W1 loads
            # threaded through the gaps (table-2/3 writes WAR-wait on the
            # gathers reading the same persistent tile, so W1 pieces slot
            # between them; all 5 W1 tiles must land by pass-0's W1 burst)
            t0d = tabs[0]["dmas"]
            DMAQ += [route_d, sel_d] + t0d + idx0
            DMAQ += tabs[1]["dmas"] + idx1 + w1dmas[:2]
            DMAQ += tabs[2]["dmas"] + idx2 + w1dmas[2:4]
            DMAQ += tabs[3]["dmas"] + w1dmas[4:] + idx3
            DMAQ += out_dmas
            chain(DMAQ)
            # Pool: pass-0 assist muls, then strict alternation with the
            # next gather ahead of the previous pass's extraction; the
            # pass-2 extraction runs BEFORE gather-3 so its reduce+W1 chain
            # clears PE before the final-pass quarters arrive
            POOLQ += tabs[0]["pool_muls"]
            POOLQ += [g1i[0], g1i[1]] + g2i[0] + [g1i[2]] + g2i[1] + g2i[2]
            POOLQ += [g1i[3]] + g2i[3]
            chain(POOLQ)
            # DVE: table muls for p+2 between scan(p) and scan(p+1); subs
            # as soon as their extraction lands; the final-pass scan chunks
            # interleave with the quarter subs
            DVEQ += tabs[0]["muls"] + tabs[1]["muls"]
            DVEQ += scans[0] + tabs[2]["muls"] + scans[1]
            DVEQ += [reds[0][0]["sub"]] + tabs[3]["muls"] + scans[2]
            DVEQ += [reds[1][0]["sub"], reds[2][0]["sub"]]
            sc3 = scans[3]
            DVEQ += [sc3[0], sc3[1], reds[3][0]["sub"], sc3[2],
                     reds[3][1]["sub"], sc3[3], reds[3][2]["sub"],
                     reds[3][3]["sub"]]
            chain(DVEQ)
            # Act: final-pass quarter bf16 downcast copies + h1 drains
            for p in range(_NTAB):
                ACTQ += [r["cp"] for r in reds[p] if r["cp"] is not None]
            ACTQ += h1copies
            chain(ACTQ)
            # PE: route matmuls in pass order; each pass's sel matmuls then
            # its W1 accumulation burst; quarters pipeline the final pass
            PEQ += tabs[0]["prs"] + tabs[1]["prs"] + tabs[2]["prs"]
            PEQ += tabs[3]["prs"]
            for p in range(_NTAB):
                for r in reds[p]:
                    PEQ += r["mms"] + r["wmms"]
            chain(PEQ)

    nc.compile()
    return nc


# ---------------------------------------------------------------- host side
def _wrap16(streams):
    """[8, J] per-group streams -> [128, J//16] wrapped-16 layout."""
    ngrp, J = streams.shape
    assert ngrp == 8 and J % 16 == 0
    out = np.zeros((_P, J // 16), streams.dtype)
    for g in range(8):
        out[g * 16 : (g + 1) * 16, :] = streams[g].reshape(J // 16, 16).T
    return out


def _core_slices(snp_ids, node_seg):
    ids = np.asarray(snp_ids).astype(np.int64)
    seg = np.asarray(node_seg).astype(np.int64)
    gpc = N_GENES // N_CORES
    gene_starts = np.searchsorted(seg, np.arange(0, N_GENES + 1, gpc))
    return ids, seg, gpc, gene_starts


def _bucket_counts(ids_c, gene_c, uniq, Kc, gpc):
    """Per-(bucket, gene) even-padded counts. bucket = T*8 + g.

    Chunks are assigned to (pass, group, half) SLOTS pairing the lightest
    chunk with the heaviest so bucket sizes (and hence the shared stream
    length J) are balanced."""
    cpos = np.searchsorted(uniq, ids_c)
    raw_chunk = cpos // Kc
    ccnt = np.bincount(raw_chunk, minlength=_NCHUNK)
    order = np.argsort(ccnt, kind="stable")  # light .. heavy
    slotof = np.empty(_NCHUNK, np.int64)
    for b in range(_NCHUNK // 2):
        T, g = b // 8, b % 8
        slotof[order[b]] = 16 * T + g  # A side
        slotof[order[_NCHUNK - 1 - b]] = 16 * T + g + 8  # B side
    cchunk = slotof[raw_chunk]
    bucketid = (cchunk // 16) * 8 + (cchunk % 8)
    key = bucketid * gpc + gene_c
    cnt = np.bincount(key, minlength=32 * gpc).reshape(32, gpc)
    pad_cnt = cnt + (cnt & 1)
    chunkof = np.empty(_NCHUNK, np.int64)
    chunkof[slotof] = np.arange(_NCHUNK)
    return cpos, cchunk, bucketid, key, cnt, pad_cnt, chunkof


def pick_cfg(snp_ids, node_seg):
    """Host pass over the indices: global compact chunk size Kc, padded
    stream length J, and the final pass's quarter extraction windows."""
    ids, seg, gpc, gene_starts = _core_slices(snp_ids, node_seg)
    Kc = 0
    uniqs = []
    for c in range(N_CORES):
        lo, hi = gene_starts[c], gene_starts[c + 1]
        uniq = np.unique(ids[lo:hi])
        uniqs.append(uniq)
        Kc = max(Kc, -(-len(uniq) // (_NCHUNK * 16)) * 16)
    J = 0
    jt = -(-gpc // _P)
    tq = jt // _NQ
    gq = [min(q * tq * _P, gpc) for q in range(_NQ)] + [gpc]
    qlo = [np.inf] * _NQ
    qhi = [0] * _NQ
    for c in range(N_CORES):
        lo, hi = gene_starts[c], gene_starts[c + 1]
        gene_c = seg[lo:hi] - c * gpc
        _, _, _, _, _, pad_cnt, _ = _bucket_counts(
            ids[lo:hi], gene_c, uniqs[c], Kc, gpc
        )
        J = max(J, 2 + int(pad_cnt.sum(axis=1).max()))
        # pass-3 buckets: pair positions of each quarter's boundary genes
        csum = 2 + np.cumsum(pad_cnt[24:32], axis=1)  # offs of gene end + pc
        ends = csum // 2 - 1  # end pair position per gene
        for q in range(_NQ):
            # boundaries used by quarter q: ends[gq[q]-1 .. gq[q+1]-1]
            # (with ends[-1] -> pair 0, always in-window)
            e_hi = ends[:, gq[q + 1] - 1].max()
            e_lo = 0 if gq[q] == 0 else ends[:, gq[q] - 1].min()
            qlo[q] = min(qlo[q], e_lo)
            qhi[q] = max(qhi[q], int(e_hi))
    J = -(-J // 16) * 16
    qwin = [(int(qlo[q]), int(qhi[q]) + 1) for q in range(_NQ)]
    return Kc, J, tuple(qwin), uniqs


def prep_inputs(cfg, snp, snp_ids, node_seg, filters, W1, uniqs):
    """Index/metadata preprocessing + zero-padding + pure layout permutation;
    all value computation happens on device."""
    import ml_dtypes

    Kc, J, gpc, gpad = cfg["Kc"], cfg["J"], cfg["gpc"], cfg["gpad"]
    nspad, d1 = cfg["nspad"], cfg["d1"]
    n_cores = cfg["n_cores"]
    gq, naq, qwin = cfg["gq"], cfg["naq"], cfg["qwin"]
    ZIDX = 2 * Kc  # zero column (even; pads point here)

    ids, seg, _, gene_starts = _core_slices(snp_ids, node_seg)
    snp = np.asarray(snp, np.float32)
    filters = np.asarray(filters, np.float32)
    W1f = np.asarray(W1, np.float32)

    # mean+replicate routing with parity zeroing: prX[m, j] =
    # (1/8) sum_r ft[s(m)X, r, j] on X-parity partitions m, 0 elsewhere.
    # ft row q = c'*8+r; routeA: c'(q) == g(m) = m//16 AND h(m) == 0,
    # routeB: c'(q) == 8+g(m) AND h(m) == 1
    route = np.zeros((_P, 2 * _P), ml_dtypes.bfloat16)
    for m in range(_P):
        g, h = m // 16, (m // 8) % 2
        if h == 0:
            route[g * 8 : g * 8 + 8, m] = 1.0 / N_FILT
        else:
            route[(8 + g) * 8 : (8 + g) * 8 + 8, _P + m] = 1.0 / N_FILT

    sel8 = np.zeros((_P, 8), ml_dtypes.bfloat16)
    for p in range(_P):
        sel8[p, p % 8] = 1.0

    per_core = []
    for c in range(n_cores):
        lo, hi = gene_starts[c], gene_starts[c + 1]
        ids_c = ids[lo:hi]
        gene_c = seg[lo:hi] - c * gpc
        uniq = uniqs[c]
        nu = len(uniq)
        assert nu <= _NCHUNK * Kc

        cpos, cchunk, bucketid, key, cnt, pad_cnt, chunkof = _bucket_counts(
            ids_c, gene_c, uniq, Kc, gpc
        )

        # compact value tables (pure permutation of inputs), arranged so
        # slot s holds chunk chunkof[s] (balanced bucket assignment)
        snp_c = np.zeros((B, _NCHUNK * Kc), np.float32)
        snp_c[:, :nu] = snp[:, uniq]
        filt_c = np.zeros((N_FILT, _NCHUNK * Kc), np.float32)
        filt_c[:, :nu] = filters[:, uniq]
        colidx = (chunkof[:, None] * Kc + np.arange(Kc)).reshape(-1)
        snp_c = snp_c[:, colidx]
        filt_c = filt_c[:, colidx]

        # PACKED table layout, pre-laid in DRAM (pure permutation): row
        # p = 16g+8h+b, pass-T block cols [T*Kc : (T+1)*Kc] hold
        # snp_c[b, (16T+g+8h)*Kc + j] (partition p's own chunk data)
        snp_perm = np.empty((_P, _NTAB * Kc), np.float32)
        sp4 = snp_perm.reshape(8, 2, 8, _NTAB * Kc)  # [g, h, b, cols]
        filt_perm = np.empty((_P, _NTAB * Kc), np.float32)
        for T in range(_NTAB):
            vi = snp_c[:, 16 * T * Kc : (16 * T + 16) * Kc].reshape(B, 2, 8, Kc)
            perm = vi.transpose(2, 1, 0, 3)  # [g, h, b, j]
            sp4[:, :, :, T * Kc : (T + 1) * Kc] = perm
            fi = filt_c[:, 16 * T * Kc : (16 * T + 16) * Kc].reshape(
                N_FILT, 16, Kc
            )
            filt_perm[:, T * Kc : (T + 1) * Kc] = fi.transpose(1, 0, 2).reshape(
                _P, Kc
            )
        filt_perm_bf = filt_perm.astype(ml_dtypes.bfloat16)

        clidx = cpos % Kc
        # gene-ordered per-bucket streams with even per-gene padding
        order = np.argsort(bucketid, kind="stable")  # gene order preserved
        skey = key[order]
        stbl = (clidx[order] + np.where((cchunk[order] % 16) >= 8, Kc, 0)).astype(
            np.int64
        )
        flat_cnt = cnt.reshape(-1)
        flat_pad = pad_cnt.reshape(-1)
        starts = np.zeros(32 * gpc, np.int64)  # node start per key
        np.cumsum(flat_cnt[:-1], out=starts[1:])
        pc = flat_pad.reshape(32, gpc)
        row_off = np.cumsum(pc, axis=1)
        offs = (
            2 + np.concatenate([np.zeros((32, 1), np.int64), row_off[:, :-1]], axis=1)
        ).reshape(-1)
        rank = np.arange(len(skey), dtype=np.int64) - starts[skey]
        pos = offs[skey] + rank
        streams = np.full((32, J), ZIDX, np.int16)
        streams[bucketid[order], pos] = stbl.astype(np.int16)
        tot = 2 + pc.sum(axis=1)
        assert int(tot.max()) <= J, f"bucket {int(tot.max())} exceeds J={J}"

        # boundaries (pair units): [0, end(g0), ..., end(g_{gpc-1})], pad;
        # pass 3 split into NQ windowed quarter streams
        ends = ((offs.reshape(32, gpc) + pc) // 2 - 1).astype(np.int64)
        ebnd = np.zeros((32, nspad), np.int16)
        ebnd[:, 1 : gpc + 1] = ends
        ebnd[:, gpc + 1 :] = ends[:, -1:].astype(np.int16)
        eqs = []
        for q in range(_NQ):
            lo_w, hi_w = qwin[q]
            ng = gq[q + 1] - gq[q]
            eq = np.zeros((8, naq[q]), np.int16)
            # col 0: previous boundary (pair 0 for q=0)
            if gq[q] == 0:
                eq[:, 0] = 0 - lo_w
            else:
                eq[:, 0] = ends[24:32, gq[q] - 1] - lo_w
            eq[:, 1 : ng + 1] = ends[24:32, gq[q] : gq[q + 1]] - lo_w
            eq[:, ng + 1 :] = eq[:, ng : ng + 1]
            assert int(eq.min()) >= 0 and int(eq.max()) < hi_w - lo_w
            eqs.append(eq)

        gidx_all = np.concatenate(
            [_wrap16(streams[T * 8 : (T + 1) * 8]) for T in range(_NTAB)], axis=1
        )
        eidx_all = np.concatenate(
            [_wrap16(ebnd[T * 8 : (T + 1) * 8]) for T in range(_NTAB - 1)]
            + [_wrap16(eq) for eq in eqs],
            axis=1,
        )

        w1c = np.zeros((gpad, d1), np.float32)
        w1c[:gpc] = W1f[c * gpc : (c + 1) * gpc]
        jt_ = gpad // _P
        w1perm = np.ascontiguousarray(
            w1c.reshape(jt_, _P, d1).transpose(1, 0, 2).reshape(_P, jt_ * d1)
        ).astype(ml_dtypes.bfloat16)

        per_core.append(
            dict(
                snp_perm=snp_perm, filt_perm=filt_perm_bf, sel=sel8, w1c=w1perm,
                mroute=route, gidx=gidx_all, eidx=eidx_all,
            )
        )
    return per_core


def host_tail(h1_sum, b1, g1, be1, W2, b2, g2, be2, W3, b3, g3, be3,
              Wh1, bh1, gh, beh, Wh2, bh2):
    def bn(x, g, be):
        return x * (g / np.sqrt(np.float32(1.0 + BN_EPS))) + be

    relu = lambda x: np.maximum(x, np.float32(0.0))
    h = relu(bn(h1_sum + b1, g1, be1))
    h = relu(bn(h @ W2 + b2, g2, be2))
    feat = relu(bn(h @ W3 + b3, g3, be3))
    m = relu(bn(feat[:, :15] @ Wh1 + bh1, gh, beh))
    return (m @ Wh2 + bh2).astype(np.float32)


_CACHE = {}


def kernel(snp, snp_ids, node_seg, filters, W1, b1, g1, be1, W2, b2, g2, be2,
           W3, b3, g3, be3, Wh1, bh1, gh, beh, Wh2, bh2):
    from concourse import bass_utils

    Kc, J, qwin, uniqs = pick_cfg(snp_ids, node_seg)
    cfg = make_cfg(Kc, J, qwin)

    key = ("v3", Kc, J, qwin)
    if key not in _CACHE:
        _CACHE[key] = build_program(cfg)
    nc = _CACHE[key]

    in_maps = prep_inputs(cfg, snp, snp_ids, node_seg, filters, W1, uniqs)
    res = bass_utils.run_bass_kernel_spmd(
        nc, in_maps, core_ids=list(range(cfg["n_cores"]))
    )
    h1_sum = np.zeros((B, cfg["d1"]), np.float32)
    for c in range(cfg["n_cores"]):
        h1_sum += res.results[c]["h1p"]

    f32 = lambda x: np.asarray(x, np.float32)
    return host_tail(h1_sum, f32(b1), f32(g1), f32(be1), f32(W2), f32(b2),
                     f32(g2), f32(be2), f32(W3), f32(b3), f32(g3), f32(be3),
                     f32(Wh1), f32(bh1), f32(gh), f32(beh), f32(Wh2), f32(bh2))


# revision 21
# speedup vs baseline: 1.0024x; 1.0024x over previous
"""Trainium2 Bass kernel for nn_AgeUGP_v2 (gnn_message_passing).

Reference pipeline:
  snp_h[b,n,f] = snp[b,n] * filters[f,n]
  gathered     = snp_h[:, snp_ids, :]
  per_gene     = segment_sum(gathered, node_seg)   # node_seg sorted
  sample_h     = per_gene.mean(-1)
  h1 = sample_h @ W1 ... tiny MLP tail

Algebraic collapse: the filter axis F is only averaged at the end, so
  sample_h[b,g] = sum_{i in seg g} snp[b, id_i] * fbar[id_i],
  fbar = mean(filters, axis=0).

Device strategy v3 (8 NeuronCores, genes sharded across cores):
  - Per-core SNP COMPACTION: each core's nodes reference ~197k unique SNPs
    (of 500k); the host selects and orders just those (pure permutation),
    split into 64 chunks of Kc.  4 table passes; pass T holds 16 chunks on
    128 partitions: partition p = 16g + 8h + b carries chunk 16T+g+8h,
    batch b.
  - PACKED table DMA + parity-zeroed route: the host ships only the raw
    chunk data [128, Kc] per pass (half the bytes of the old zero-split
    layout); it lands in the B-half region [Kc:2Kc) of a persistent wide
    table [128, 2Kc+2].  fbar is produced fused on device: bf16 filters are
    hit with 1/8-valued mean+replicate PE matmuls whose routeA (routeB)
    matrices are ZERO on h=1 (h=0) partition columns.  The A-half multiply
    vtab[:,0:Kc) = raw * prA then writes data*fbar on h=0 lanes and EXACT 0
    on h=1 lanes; the B-half multiply runs in place.  The resulting table
    has the zero-split property (an index in [0,Kc) reads chunk A's value
    on h=0 lanes and 0 on h=1 lanes) without shipping or memsetting zeros
    (the two trailing zero columns are memset once).
  - One gpsimd ap_gather per pass streams both chunks' nodes gene-ordered
    (per-gene counts padded to EVEN with pads pointing at the zero column).
    A DVE tensor_tensor_scan with data0/data1 = even/odd stride-2 views
    forms PAIR prefix sums in place; a second ap_gather extracts one prefix
    per gene END; one adjacent difference gives per-(gene,half,batch) sums
    in dd (bf16); sel matmuls fold halves+lanes into pst [gene, batch].
  - PER-PASS W1 ACCUMULATION: each pass's partial pst is copied to bf16 on
    the otherwise-idle Activation engine and immediately matmul'd with the
    resident W1 shard, accumulating in two PSUM banks across all 4 passes
    (h1 = sum_p partial_p @ W1).  The old end-of-kernel W1 burst is gone.
  - The FINAL pass's extraction is split into 4 gene-quarter windows so
    the last sub/selmm/W1 chain covers only ~1/4 of the genes.
  - Pass-0 table multiplies are split DVE/Pool so the first gather starts
    ~4us earlier (Pool is idle in the head anyway).
Scheduling: the Tile scheduler's internal timing model mispredicts DMA
completion, so the per-engine order is fully pinned with nosync chains.
"""

import numpy as np

B = 8
N_SNPS = 500000
N_NODES = 2000000
N_GENES = 20000
N_FILT = 8
N_CORES = 8
BN_EPS = 1e-5

_P = 128
_NCHUNK = 64  # compact SNP chunks per core
_NTAB = 4  # table passes
_EPAD = 16
_NQ = 4  # gene quarters for the final pass's split extraction


def make_cfg(Kc, J, qwin=None, n_genes=N_GENES, n_cores=N_CORES, d1=1024):
    gpc = n_genes // n_cores
    jt = -(-gpc // _P)
    gpad = jt * _P
    ns = gpc + 1  # boundaries: dummy zero + one end per gene
    nspad = -(-ns // _EPAD) * _EPAD
    # final pass: gene quarters on sel-tile boundaries (tiles of 128 genes)
    tq = jt // _NQ
    gq = [min(q * tq * _P, gpc) for q in range(_NQ)] + [gpc]
    naq = [-(-(gq[q + 1] - gq[q] + 1) // _EPAD) * _EPAD for q in range(_NQ)]
    assert J % 16 == 0 and J % 4 == 0
    assert 2 * Kc + 2 <= 2**15, "gather table exceeds num_elems limit"
    assert J <= 32752, "stream length exceeds int16 index range"
    return dict(
        Kc=Kc, J=J, gpc=gpc, gpad=gpad, jt=jt, d1=d1, ns=ns, nspad=nspad,
        n_cores=n_cores, gq=gq, naq=naq, qwin=qwin,
    )


# ---------------------------------------------------------------- device program
def build_program(cfg):
    import concourse.bass as bass
    import concourse.bacc as bacc
    import concourse.mybir as mybir
    import concourse.tile as tile

    fp32 = mybir.dt.float32
    bf16 = mybir.dt.bfloat16
    i16 = mybir.dt.int16

    Kc, J = cfg["Kc"], cfg["J"]
    jt, d1 = cfg["jt"], cfg["d1"]
    gpc, gpad, nspad = cfg["gpc"], cfg["gpad"], cfg["nspad"]
    gq, naq, qwin = cfg["gq"], cfg["naq"], cfg["qwin"]
    TW = 2 * Kc + 2  # table width: [A-half | B-half | zero col pair]
    JH = J // 2
    NS2 = sum(naq)

    nc = bacc.Bacc(
        "TRN2", target_bir_lowering=False, debug=False, num_devices=cfg["n_cores"]
    )

    snp_in = nc.dram_tensor("snp_perm", [_P, _NTAB * Kc], fp32, kind="ExternalInput")
    filt_in = nc.dram_tensor("filt_perm", [_P, _NTAB * Kc], bf16, kind="ExternalInput")
    gidx_in = nc.dram_tensor("gidx", [_P, _NTAB * (J // 16)], i16, kind="ExternalInput")
    eidx_in = nc.dram_tensor(
        "eidx", [_P, (_NTAB - 1) * (nspad // 16) + NS2 // 16], i16,
        kind="ExternalInput",
    )
    sel_in = nc.dram_tensor("sel", [_P, 8], bf16, kind="ExternalInput")
    route_in = nc.dram_tensor("mroute", [_P, 2 * _P], bf16, kind="ExternalInput")
    w1_in = nc.dram_tensor("w1c", [_P, jt * d1], bf16, kind="ExternalInput")
    h1_out = nc.dram_tensor("h1p", [B, d1], fp32, kind="ExternalOutput")

    rc = Kc // 8  # route/mul block width (single-bank PSUM tiles)
    assert rc * 8 == Kc and rc <= 512

    with tile.TileContext(nc) as tc:
        with (
            tc.tile_pool(name="per", bufs=1) as perpool,
            tc.tile_pool(name="gs", bufs=2) as gspool,
            tc.tile_pool(name="ft", bufs=2) as ftpool,
            tc.tile_pool(name="ex", bufs=2) as expool,
            tc.tile_pool(name="ixg", bufs=3) as ixgpool,
            tc.tile_pool(name="ixe", bufs=3) as ixepool,
            tc.tile_pool(name="w1", bufs=5) as w1pool,
            tc.tile_pool(name="shb", bufs=2) as shbpool,
            tc.tile_pool(name="ps", bufs=5, space="PSUM") as pspool,
            tc.tile_pool(name="psw", bufs=1, space="PSUM") as pswpool,
            tc.tile_pool(name="psh", bufs=1, space="PSUM") as pshpool,
        ):
            DMAQ, DVEQ, POOLQ, PEQ, ACTQ = [], [], [], [], []

            route = perpool.tile([_P, 2 * _P], bf16, tag="route")
            route_d = nc.sync.dma_start(route[:], route_in.ap())
            sel8 = perpool.tile([_P, 8], bf16, tag="sel8")
            sel_d = nc.sync.dma_start(sel8[:], sel_in.ap())

            # dd holds per-(lane,gene) sums; pad cols stay zero forever
            dd = perpool.tile([_P, gpad], bf16, tag="dd")
            DVEQ.append(nc.vector.memset(dd[:], 0.0))
            # two persistent wide tables; only the trailing zero column pair
            # needs initialization (the A/B data halves are regenerated by
            # the parity-zeroed multiplies each pass)
            vtabs_b = []
            for t in range(2):
                vt = perpool.tile([_P, TW], fp32, tag=f"vtab{t}")
                DVEQ.append(nc.vector.memset(vt[:, 2 * Kc : TW], 0.0))
                vtabs_b.append(vt)

            from concourse.instruction_name_ordered_set import (
                InstructionNameOrderedSet,
            )

            def pin(later, earlier):
                """Same-engine order pin (no runtime semaphore)."""
                s = InstructionNameOrderedSet()
                s.add(earlier.ins.name)
                later.ins.add_nosync_dependencies_from(s)

            def emit_table(T, pool_blocks=()):
                # raw packed data lands in the B-half region [Kc:2Kc); the
                # A-half multiply reads it (cross-region), the B-half
                # multiply runs in place.  prA/prB are zero on wrong-parity
                # partitions, so both halves get the zero-split layout
                # without any zero DMA/memset.  pool_blocks run their
                # multiplies on the (head-idle) gpsimd engine; their prs are
                # emitted first so the PE queue serves them first.
                vtab = vtabs_b[T % 2]
                ft = ftpool.tile([_P, Kc], bf16, tag="ftl", name=f"ftl{T}")
                ftd = nc.sync.dma_start(
                    ft[:], filt_in.ap()[:, T * Kc : (T + 1) * Kc]
                )
                if T == 0:
                    # lead-in: quarter the transfer so multiplies pipeline
                    KH = Kc // 4
                    qs = [
                        nc.sync.dma_start(
                            vtab[:, Kc + q * KH : Kc + (q + 1) * KH],
                            snp_in.ap()[:, q * KH : (q + 1) * KH],
                        )
                        for q in range(4)
                    ]
                    dmas = [ftd] + qs
                else:
                    dmas = [
                        ftd,
                        nc.sync.dma_start(
                            vtab[:, Kc : 2 * Kc],
                            snp_in.ap()[:, T * Kc : (T + 1) * Kc],
                        ),
                    ]
                dve_muls, pool_muls, prs = [], [], []
                blk_order = [
                    b for b in range(8) if b not in pool_blocks
                ] + list(pool_blocks)
                for blk in blk_order:
                    rs = slice(Kc + blk * rc, Kc + (blk + 1) * rc)
                    on_pool = blk in pool_blocks
                    for half in range(2):
                        pr = pspool.tile([_P, rc], fp32, tag="pr", name="pr")
                        prs.append(
                            nc.tensor.matmul(
                                pr[:],
                                route[:, half * _P : (half + 1) * _P],
                                ft[:, blk * rc : (blk + 1) * rc],
                                start=True, stop=True,
                            )
                        )
                        ks = rs if half else slice(blk * rc, (blk + 1) * rc)
                        eng = nc.gpsimd if on_pool else nc.vector
                        m = eng.tensor_mul(vtab[:, ks], vtab[:, rs], pr[:])
                        (pool_muls if on_pool else dve_muls).append(m)
                vtabs_for_pass[T] = vtab
                return dict(dmas=dmas, muls=dve_muls, pool_muls=pool_muls, prs=prs)

            vtabs_for_pass = {}

            # index streams prefetched once (each pass's stream is its own
            # tile: ap_gather idx APs must start at a tile base)
            gidx_t, eidx_t = {}, {}

            def prefetch_gidx(p):
                g = ixgpool.tile([_P, J // 16], i16, tag="gidxp", name=f"gidx{p}")
                d = nc.sync.dma_start(
                    g[:], gidx_in.ap()[:, p * (J // 16) : (p + 1) * (J // 16)]
                )
                gidx_t[p] = g
                return [d]

            def prefetch_eidx(p):
                dmas = []
                if p < _NTAB - 1:
                    e = ixepool.tile([_P, nspad // 16], i16, tag="eidxp",
                                     name=f"eidx{p}")
                    dmas.append(
                        nc.sync.dma_start(
                            e[:],
                            eidx_in.ap()[
                                :, p * (nspad // 16) : (p + 1) * (nspad // 16)
                            ],
                        )
                    )
                    eidx_t[p] = e
                else:
                    base3 = (_NTAB - 1) * (nspad // 16)
                    eqs = []
                    off = base3
                    for q in range(_NQ):
                        eq = perpool.tile([_P, naq[q] // 16], i16, tag=f"eidxq{q}")
                        dmas.append(
                            nc.sync.dma_start(
                                eq[:], eidx_in.ap()[:, off : off + naq[q] // 16]
                            )
                        )
                        eqs.append(eq)
                        off += naq[q] // 16
                    eidx_t[p] = eqs
                return dmas

            def emit_gather(p):
                gidx = gidx_t[p]
                gout = gspool.tile([_P, J], fp32, tag="gout", name=f"gout{p}")
                g1 = nc.gpsimd.ap_gather(
                    gout[:], vtabs_for_pass.pop(p)[:], gidx[:],
                    channels=_P, num_elems=TW, d=1, num_idxs=J,
                )
                return gout, g1

            def _scan_piece(gout, c0, c1, initial):
                # pair prefix over stream slots [2*c0, 2*c1) into pair cols
                # [c0, c1), chained via `initial`
                ge = gout[:, 2 * c0 :]
                even = bass.AP(ge.tensor, ge.offset, [ge.ap[0], [2, c1 - c0]])
                go = gout[:, 2 * c0 + 1 :]
                odd = bass.AP(go.tensor, go.offset, [go.ap[0], [2, c1 - c0]])
                return nc.vector.tensor_tensor_scan(
                    gout[:, c0:c1], even, odd, initial,
                    op0=mybir.AluOpType.add, op1=mybir.AluOpType.add,
                )

            def emit_scan_extract(p, gout):
                # pair prefix scan, in place into the first half (writes
                # trail the stride-2 reads)
                if p < _NTAB - 1:
                    sc = [_scan_piece(gout, 0, JH, 0.0)]
                    ext = expool.tile([_P, nspad], fp32, tag="ex", name=f"ex{p}")
                    eidx = eidx_t[p]
                    g2 = [
                        nc.gpsimd.ap_gather(
                            ext[:], gout[:, :JH], eidx[:],
                            channels=_P, num_elems=JH, d=1, num_idxs=nspad,
                        )
                    ]
                    return sc, g2, [ext]
                # final pass: the scan is chunked at the quarter windows'
                # upper bounds and each quarter extracts into its OWN tile,
                # so quarter q's reduce chain starts as soon as scan chunk q
                # and its (windowed) extraction are done
                eqs = eidx_t[p]
                sc, g2, exts = [], [], []
                c0 = 0
                for q in range(_NQ):
                    c1 = qwin[q][1] if q < _NQ - 1 else JH
                    sc.append(
                        _scan_piece(gout, c0, c1,
                                    0.0 if q == 0 else gout[:, c0 - 1 : c0])
                    )
                    lo, hi = qwin[q]
                    ext = expool.tile([_P, naq[q]], fp32, tag="ex",
                                      name=f"exq{q}")
                    g2.append(
                        nc.gpsimd.ap_gather(
                            ext[:], gout[:, lo:hi], eqs[q][:],
                            channels=_P, num_elems=hi - lo, d=1,
                            num_idxs=naq[q],
                        )
                    )
                    exts.append(ext)
                    c0 = c1
                return sc, g2, exts

            # ---- reduce: sel matmuls ACCUMULATE all 4 passes into one
            # persistent PSUM tile (start on pass 0, stop in the final
            # pass's quarter); W1 then runs once, per gene quarter --------
            n_half = min(512, d1)
            n_banks = -(-d1 // n_half)
            pss = []
            for nb in range(n_banks):
                ps = pswpool.tile([_P, n_half], fp32, tag=f"ps{nb}", name=f"ps{nb}")
                pss.append(ps)
            pst = pshpool.tile([_P, jt * B], fp32, tag="pst")

            w1ts, w1dmas = [], []
            wgrp = 4 if jt % 4 == 0 else 1  # K-tiles per W1 load
            for jg in range(jt // wgrp):
                w1t = w1pool.tile([_P, wgrp * d1], bf16, tag="w1t", name=f"w1t{jg}")
                w1dmas.append(
                    nc.sync.dma_start(
                        w1t[:], w1_in.ap()[:, jg * wgrp * d1 : (jg + 1) * wgrp * d1]
                    )
                )
                w1ts.append(w1t)

            def w1_tile(j):
                return w1ts[j // wgrp], (j % wgrp) * d1

            def emit_reduce(p, ex, tlo, thi, xoff, first, last):
                """sub -> sel matmuls accumulating pst[:, tile cols]; on the
                final pass (last=True) also bf16-copy the finished quarter
                and run its W1 matmuls (accumulating into pss banks)."""
                glo, ghi = tlo * _P, min(thi * _P, gpc)
                sub = nc.vector.tensor_sub(
                    dd[:, glo:ghi],
                    ex[:, xoff + 1 : xoff + 1 + (ghi - glo)],
                    ex[:, xoff : xoff + (ghi - glo)],
                )
                mms = []
                for t in range(tlo, thi):
                    mms.append(
                        nc.tensor.matmul(
                            pst[:, t * B : (t + 1) * B],
                            dd[:, t * _P : (t + 1) * _P],
                            sel8[:],
                            start=first, stop=last,
                        )
                    )
                if not last:
                    return dict(sub=sub, mms=mms, cp=None, wmms=[])
                shb = shbpool.tile([_P, (thi - tlo) * B], bf16, tag="shbp",
                                   name=f"shbq{tlo}")
                cp = nc.scalar.copy(shb[:], pst[:, tlo * B : thi * B])
                wmms = []
                for nb in range(n_banks):  # bank-major: bank0 stops first
                    for i, t in enumerate(range(tlo, thi)):
                        w1t, woff = w1_tile(t)
                        wmms.append(
                            nc.tensor.matmul(
                                pss[nb][:B, :],
                                shb[:, i * B : (i + 1) * B],
                                w1t[:, woff + nb * n_half : woff + (nb + 1) * n_half],
                                start=tlo == 0 and t == tlo,
                                stop=t == jt - 1,
                            )
                        )
                return dict(sub=sub, mms=mms, cp=cp, wmms=wmms)

            # ---------------- emission (dataflow order: the tile
            # framework's auto-deps are derived from emission order, so
            # writers of reused tiles must be emitted after their prior
            # readers; DMAQ/engine pins control actual timing) ----------
            tabs = {0: emit_table(0, pool_blocks=(6, 7))}
            idx0 = prefetch_gidx(0)
            tabs[1] = emit_table(1)
            idx1 = prefetch_gidx(1) + prefetch_eidx(0)
            idx2 = prefetch_gidx(2) + prefetch_eidx(1)
            idx3 = prefetch_eidx(2) + prefetch_eidx(3)

            gouts, g1i, scans, g2i, exs = {}, {}, {}, {}, {}
            reds = {}
            gouts[0], g1i[0] = emit_gather(0)
            for p in range(_NTAB):
                if p == 0:
                    # gidx3 reuses gidx0's pool slot: emit after gather-0
                    idx3 = prefetch_gidx(3) + idx3
                if p + 1 < _NTAB:
                    gouts[p + 1], g1i[p + 1] = emit_gather(p + 1)
                scans[p], g2i[p], exs[p] = emit_scan_extract(p, gouts.pop(p))
                if p + 2 < _NTAB:
                    tabs[p + 2] = emit_table(p + 2)
                if p >= 1:
                    pp = p - 1
                    reds[pp] = [emit_reduce(pp, exs.pop(pp)[0], 0, jt, 0,
                                            first=pp == 0, last=False)]
            # final pass: per-quarter reduce chains
            ex3s = exs.pop(_NTAB - 1)
            reds[_NTAB - 1] = []
            tq = jt // _NQ
            for q in range(_NQ):
                reds[_NTAB - 1].append(
                    emit_reduce(_NTAB - 1, ex3s[q], q * tq,
                                min((q + 1) * tq, jt), 0,
                                first=False, last=True)
                )

            # PE warm-up matmuls: bridge the PE-idle window between the
            # last per-pass sel matmuls and the final-pass quarter bursts
            # so the W1 matmuls run at the full-clock p-state
            warms = []
            for w in range(30):
                wt = pspool.tile([_P, rc], fp32, tag="pr", name=f"warm{w}")
                warms.append(
                    nc.tensor.matmul(
                        wt[:], route[:, 0:_P], w1ts[0][:, 0:rc],
                        start=True, stop=True,
                    )
                )

            # per-bank copy + output DMA so bank 0's drain overlaps bank 1's
            # matmuls
            h1 = perpool.tile([B, d1], fp32, tag="h1")
            h1copies, out_dmas = [], []
            for nb in range(n_banks):
                h1copies.append(
                    nc.scalar.copy(
                        h1[:, nb * n_half : (nb + 1) * n_half], pss[nb][:B, :]
                    )
                )
                out_dmas.append(
                    nc.sync.dma_start(
                        h1_out.ap()[:, nb * n_half : (nb + 1) * n_half],
                        h1[:, nb * n_half : (nb + 1) * n_half],
                    )
                )

            # ---------------- static order pins ----------------
            def chain(seq):
                for a, b in zip(seq, seq[1:]):
                    pin(b, a)

            # DMA: route/sel tiny and first (they gate the route matmuls),
            # table 0 (ft first), gather-0 indices, table 1, then W1 loads
            # threaded through the gaps (table-2/3 writes WAR-wait on the
            # gathers reading the same persistent tile, so W1 pieces slot
            # between them; all 5 W1 tiles must land by pass-0's W1 burst)
            t0d = tabs[0]["dmas"]
            DMAQ += [route_d, sel_d] + t0d + idx0
            DMAQ += tabs[1]["dmas"] + idx1 + w1dmas[:2]
            DMAQ += tabs[2]["dmas"] + idx2 + w1dmas[2:4]
            DMAQ += tabs[3]["dmas"] + w1dmas[4:] + idx3
            DMAQ += out_dmas
            chain(DMAQ)
            # Pool: pass-0 assist muls, then strict alternation with the
            # next gather ahead of the previous pass's extraction; the
            # pass-2 extraction runs BEFORE gather-3 so its reduce+W1 chain
            # clears PE before the final-pass quarters arrive
            POOLQ += tabs[0]["pool_muls"]
            POOLQ += [g1i[0], g1i[1]] + g2i[0] + [g1i[2]] + g2i[1] + g2i[2]
            POOLQ += [g1i[3]] + g2i[3]
            chain(POOLQ)
            # DVE: table muls for p+2 between scan(p) and scan(p+1); subs
            # as soon as their extraction lands; the final-pass scan chunks
            # interleave with the quarter subs
            DVEQ += tabs[0]["muls"] + tabs[1]["muls"]
            DVEQ += scans[0] + tabs[2]["muls"] + scans[1]
            DVEQ += tabs[3]["muls"] + scans[2]
            DVEQ += [reds[0][0]["sub"], reds[1][0]["sub"], reds[2][0]["sub"]]
            sc3 = scans[3]
            DVEQ += [sc3[0], sc3[1], reds[3][0]["sub"], sc3[2],
                     reds[3][1]["sub"], sc3[3], reds[3][2]["sub"],
                     reds[3][3]["sub"]]
            chain(DVEQ)
            # Act: final-pass quarter bf16 downcast copies + h1 drains
            for p in range(_NTAB):
                ACTQ += [r["cp"] for r in reds[p] if r["cp"] is not None]
            ACTQ += h1copies
            chain(ACTQ)
            # PE: route matmuls in pass order; each pass's sel matmuls then
            # its W1 accumulation burst; quarters pipeline the final pass
            PEQ += tabs[0]["prs"] + tabs[1]["prs"] + tabs[2]["prs"]
            PEQ += tabs[3]["prs"]
            for p in range(_NTAB - 1):
                for r in reds[p]:
                    PEQ += r["mms"] + r["wmms"]
            PEQ += warms
            for r in reds[_NTAB - 1]:
                PEQ += r["mms"] + r["wmms"]
            chain(PEQ)

    nc.compile()
    return nc


# ---------------------------------------------------------------- host side
def _wrap16(streams):
    """[8, J] per-group streams -> [128, J//16] wrapped-16 layout."""
    ngrp, J = streams.shape
    assert ngrp == 8 and J % 16 == 0
    out = np.zeros((_P, J // 16), streams.dtype)
    for g in range(8):
        out[g * 16 : (g + 1) * 16, :] = streams[g].reshape(J // 16, 16).T
    return out


def _core_slices(snp_ids, node_seg):
    ids = np.asarray(snp_ids).astype(np.int64)
    seg = np.asarray(node_seg).astype(np.int64)
    gpc = N_GENES // N_CORES
    gene_starts = np.searchsorted(seg, np.arange(0, N_GENES + 1, gpc))
    return ids, seg, gpc, gene_starts


def _bucket_counts(ids_c, gene_c, uniq, Kc, gpc):
    """Per-(bucket, gene) even-padded counts. bucket = T*8 + g.

    Chunks are assigned to (pass, group, half) SLOTS pairing the lightest
    chunk with the heaviest so bucket sizes (and hence the shared stream
    length J) are balanced."""
    cpos = np.searchsorted(uniq, ids_c)
    raw_chunk = cpos // Kc
    ccnt = np.bincount(raw_chunk, minlength=_NCHUNK)
    order = np.argsort(ccnt, kind="stable")  # light .. heavy
    slotof = np.empty(_NCHUNK, np.int64)
    for b in range(_NCHUNK // 2):
        T, g = b // 8, b % 8
        slotof[order[b]] = 16 * T + g  # A side
        slotof[order[_NCHUNK - 1 - b]] = 16 * T + g + 8  # B side
    cchunk = slotof[raw_chunk]
    bucketid = (cchunk // 16) * 8 + (cchunk % 8)
    key = bucketid * gpc + gene_c
    cnt = np.bincount(key, minlength=32 * gpc).reshape(32, gpc)
    pad_cnt = cnt + (cnt & 1)
    chunkof = np.empty(_NCHUNK, np.int64)
    chunkof[slotof] = np.arange(_NCHUNK)
    return cpos, cchunk, bucketid, key, cnt, pad_cnt, chunkof


def pick_cfg(snp_ids, node_seg):
    """Host pass over the indices: global compact chunk size Kc, padded
    stream length J, and the final pass's quarter extraction windows."""
    ids, seg, gpc, gene_starts = _core_slices(snp_ids, node_seg)
    Kc = 0
    uniqs = []
    for c in range(N_CORES):
        lo, hi = gene_starts[c], gene_starts[c + 1]
        uniq = np.unique(ids[lo:hi])
        uniqs.append(uniq)
        Kc = max(Kc, -(-len(uniq) // (_NCHUNK * 16)) * 16)
    J = 0
    jt = -(-gpc // _P)
    tq = jt // _NQ
    gq = [min(q * tq * _P, gpc) for q in range(_NQ)] + [gpc]
    qlo = [np.inf] * _NQ
    qhi = [0] * _NQ
    for c in range(N_CORES):
        lo, hi = gene_starts[c], gene_starts[c + 1]
        gene_c = seg[lo:hi] - c * gpc
        _, _, _, _, _, pad_cnt, _ = _bucket_counts(
            ids[lo:hi], gene_c, uniqs[c], Kc, gpc
        )
        J = max(J, 2 + int(pad_cnt.sum(axis=1).max()))
        # pass-3 buckets: pair positions of each quarter's boundary genes
        csum = 2 + np.cumsum(pad_cnt[24:32], axis=1)  # offs of gene end + pc
        ends = csum // 2 - 1  # end pair position per gene
        for q in range(_NQ):
            # boundaries used by quarter q: ends[gq[q]-1 .. gq[q+1]-1]
            # (with ends[-1] -> pair 0, always in-window)
            e_hi = ends[:, gq[q + 1] - 1].max()
            e_lo = 0 if gq[q] == 0 else ends[:, gq[q] - 1].min()
            qlo[q] = min(qlo[q], e_lo)
            qhi[q] = max(qhi[q], int(e_hi))
    J = -(-J // 16) * 16
    qwin = [(int(qlo[q]), int(qhi[q]) + 1) for q in range(_NQ)]
    return Kc, J, tuple(qwin), uniqs


def prep_inputs(cfg, snp, snp_ids, node_seg, filters, W1, uniqs):
    """Index/metadata preprocessing + zero-padding + pure layout permutation;
    all value computation happens on device."""
    import ml_dtypes

    Kc, J, gpc, gpad = cfg["Kc"], cfg["J"], cfg["gpc"], cfg["gpad"]
    nspad, d1 = cfg["nspad"], cfg["d1"]
    n_cores = cfg["n_cores"]
    gq, naq, qwin = cfg["gq"], cfg["naq"], cfg["qwin"]
    ZIDX = 2 * Kc  # zero column (even; pads point here)

    ids, seg, _, gene_starts = _core_slices(snp_ids, node_seg)
    snp = np.asarray(snp, np.float32)
    filters = np.asarray(filters, np.float32)
    W1f = np.asarray(W1, np.float32)

    # mean+replicate routing with parity zeroing: prX[m, j] =
    # (1/8) sum_r ft[s(m)X, r, j] on X-parity partitions m, 0 elsewhere.
    # ft row q = c'*8+r; routeA: c'(q) == g(m) = m//16 AND h(m) == 0,
    # routeB: c'(q) == 8+g(m) AND h(m) == 1
    route = np.zeros((_P, 2 * _P), ml_dtypes.bfloat16)
    for m in range(_P):
        g, h = m // 16, (m // 8) % 2
        if h == 0:
            route[g * 8 : g * 8 + 8, m] = 1.0 / N_FILT
        else:
            route[(8 + g) * 8 : (8 + g) * 8 + 8, _P + m] = 1.0 / N_FILT

    sel8 = np.zeros((_P, 8), ml_dtypes.bfloat16)
    for p in range(_P):
        sel8[p, p % 8] = 1.0

    per_core = []
    for c in range(n_cores):
        lo, hi = gene_starts[c], gene_starts[c + 1]
        ids_c = ids[lo:hi]
        gene_c = seg[lo:hi] - c * gpc
        uniq = uniqs[c]
        nu = len(uniq)
        assert nu <= _NCHUNK * Kc

        cpos, cchunk, bucketid, key, cnt, pad_cnt, chunkof = _bucket_counts(
            ids_c, gene_c, uniq, Kc, gpc
        )

        # compact value tables (pure permutation of inputs), arranged so
        # slot s holds chunk chunkof[s] (balanced bucket assignment)
        snp_c = np.zeros((B, _NCHUNK * Kc), np.float32)
        snp_c[:, :nu] = snp[:, uniq]
        filt_c = np.zeros((N_FILT, _NCHUNK * Kc), np.float32)
        filt_c[:, :nu] = filters[:, uniq]
        colidx = (chunkof[:, None] * Kc + np.arange(Kc)).reshape(-1)
        snp_c = snp_c[:, colidx]
        filt_c = filt_c[:, colidx]

        # PACKED table layout, pre-laid in DRAM (pure permutation): row
        # p = 16g+8h+b, pass-T block cols [T*Kc : (T+1)*Kc] hold
        # snp_c[b, (16T+g+8h)*Kc + j] (partition p's own chunk data)
        snp_perm = np.empty((_P, _NTAB * Kc), np.float32)
        sp4 = snp_perm.reshape(8, 2, 8, _NTAB * Kc)  # [g, h, b, cols]
        filt_perm = np.empty((_P, _NTAB * Kc), np.float32)
        for T in range(_NTAB):
            vi = snp_c[:, 16 * T * Kc : (16 * T + 16) * Kc].reshape(B, 2, 8, Kc)
            perm = vi.transpose(2, 1, 0, 3)  # [g, h, b, j]
            sp4[:, :, :, T * Kc : (T + 1) * Kc] = perm
            fi = filt_c[:, 16 * T * Kc : (16 * T + 16) * Kc].reshape(
                N_FILT, 16, Kc
            )
            filt_perm[:, T * Kc : (T + 1) * Kc] = fi.transpose(1, 0, 2).reshape(
                _P, Kc
            )
        filt_perm_bf = filt_perm.astype(ml_dtypes.bfloat16)

        clidx = cpos % Kc
        # gene-ordered per-bucket streams with even per-gene padding
        order = np.argsort(bucketid, kind="stable")  # gene order preserved
        skey = key[order]
        stbl = (clidx[order] + np.where((cchunk[order] % 16) >= 8, Kc, 0)).astype(
            np.int64
        )
        flat_cnt = cnt.reshape(-1)
        flat_pad = pad_cnt.reshape(-1)
        starts = np.zeros(32 * gpc, np.int64)  # node start per key
        np.cumsum(flat_cnt[:-1], out=starts[1:])
        pc = flat_pad.reshape(32, gpc)
        row_off = np.cumsum(pc, axis=1)
        offs = (
            2 + np.concatenate([np.zeros((32, 1), np.int64), row_off[:, :-1]], axis=1)
        ).reshape(-1)
        rank = np.arange(len(skey), dtype=np.int64) - starts[skey]
        pos = offs[skey] + rank
        streams = np.full((32, J), ZIDX, np.int16)
        streams[bucketid[order], pos] = stbl.astype(np.int16)
        tot = 2 + pc.sum(axis=1)
        assert int(tot.max()) <= J, f"bucket {int(tot.max())} exceeds J={J}"

        # boundaries (pair units): [0, end(g0), ..., end(g_{gpc-1})], pad;
        # pass 3 split into NQ windowed quarter streams
        ends = ((offs.reshape(32, gpc) + pc) // 2 - 1).astype(np.int64)
        ebnd = np.zeros((32, nspad), np.int16)
        ebnd[:, 1 : gpc + 1] = ends
        ebnd[:, gpc + 1 :] = ends[:, -1:].astype(np.int16)
        eqs = []
        for q in range(_NQ):
            lo_w, hi_w = qwin[q]
            ng = gq[q + 1] - gq[q]
            eq = np.zeros((8, naq[q]), np.int16)
            # col 0: previous boundary (pair 0 for q=0)
            if gq[q] == 0:
                eq[:, 0] = 0 - lo_w
            else:
                eq[:, 0] = ends[24:32, gq[q] - 1] - lo_w
            eq[:, 1 : ng + 1] = ends[24:32, gq[q] : gq[q + 1]] - lo_w
            eq[:, ng + 1 :] = eq[:, ng : ng + 1]
            assert int(eq.min()) >= 0 and int(eq.max()) < hi_w - lo_w
            eqs.append(eq)

        gidx_all = np.concatenate(
            [_wrap16(streams[T * 8 : (T + 1) * 8]) for T in range(_NTAB)], axis=1
        )
        eidx_all = np.concatenate(
            [_wrap16(ebnd[T * 8 : (T + 1) * 8]) for T in range(_NTAB - 1)]
            + [_wrap16(eq) for eq in eqs],
            axis=1,
        )

        w1c = np.zeros((gpad, d1), np.float32)
        w1c[:gpc] = W1f[c * gpc : (c + 1) * gpc]
        jt_ = gpad // _P
        w1perm = np.ascontiguousarray(
            w1c.reshape(jt_, _P, d1).transpose(1, 0, 2).reshape(_P, jt_ * d1)
        ).astype(ml_dtypes.bfloat16)

        per_core.append(
            dict(
                snp_perm=snp_perm, filt_perm=filt_perm_bf, sel=sel8, w1c=w1perm,
                mroute=route, gidx=gidx_all, eidx=eidx_all,
            )
        )
    return per_core


def host_tail(h1_sum, b1, g1, be1, W2, b2, g2, be2, W3, b3, g3, be3,
              Wh1, bh1, gh, beh, Wh2, bh2):
    def bn(x, g, be):
        return x * (g / np.sqrt(np.float32(1.0 + BN_EPS))) + be

    relu = lambda x: np.maximum(x, np.float32(0.0))
    h = relu(bn(h1_sum + b1, g1, be1))
    h = relu(bn(h @ W2 + b2, g2, be2))
    feat = relu(bn(h @ W3 + b3, g3, be3))
    m = relu(bn(feat[:, :15] @ Wh1 + bh1, gh, beh))
    return (m @ Wh2 + bh2).astype(np.float32)


_CACHE = {}


def kernel(snp, snp_ids, node_seg, filters, W1, b1, g1, be1, W2, b2, g2, be2,
           W3, b3, g3, be3, Wh1, bh1, gh, beh, Wh2, bh2):
    from concourse import bass_utils

    Kc, J, qwin, uniqs = pick_cfg(snp_ids, node_seg)
    cfg = make_cfg(Kc, J, qwin)

    key = ("v3", Kc, J, qwin)
    if key not in _CACHE:
        _CACHE[key] = build_program(cfg)
    nc = _CACHE[key]

    in_maps = prep_inputs(cfg, snp, snp_ids, node_seg, filters, W1, uniqs)
    res = bass_utils.run_bass_kernel_spmd(
        nc, in_maps, core_ids=list(range(cfg["n_cores"]))
    )
    h1_sum = np.zeros((B, cfg["d1"]), np.float32)
    for c in range(cfg["n_cores"]):
        h1_sum += res.results[c]["h1p"]

    f32 = lambda x: np.asarray(x, np.float32)
    return host_tail(h1_sum, f32(b1), f32(g1), f32(be1), f32(W2), f32(b2),
                     f32(g2), f32(be2), f32(W3), f32(b3), f32(g3), f32(be3),
                     f32(Wh1), f32(bh1), f32(gh), f32(beh), f32(Wh2), f32(bh2))
